# revision 1
# baseline (speedup 1.0000x reference)
"""Dense CRF mean-field inference (2 labels) on 8 Trainium2 NeuronCores.

Strategy (column-sharded, collective-synchronized):
  - N = 80*80 = 6400 pixels. Core c owns the contiguous i-block
    I_c = [c*800, (c+1)*800).
  - Both pairwise kernels are Gaussians of squared feature distances, so
    K[j,i] = exp(f_j.f_i - |f_i|^2/2 - |f_j|^2/2) is built on the tensor
    engine via an augmented-feature gram matmul ([f_i; -|f_i|^2/2] against
    [f_j; 1]), with the per-j offset applied as the activation bias of the
    Exp that materializes each [128, 800] tile. Tiles are stored in HBM
    (fp16 by default), [50, 128, 800] per kernel (Kg, Kb img0, Kb img1).
  - Row sums for the symmetric normalization fall out of the Exp's
    accum_out (free-axis sum = partial sum over the core's i-block);
    an AllReduce produces full row sums (j-layout) and a ReduceScatter on
    a rank-major reshuffle hands each core the sums for its own i-block.
  - Each of the `iters` mean-field steps streams the 3 kernel blocks from
    HBM once: out[l, i] += (Q * n_j)^T-tile-stationary matmuls accumulated
    over all 50 j-tiles into PSUM; a PE transpose flips messages to
    [i-partition, l] layout for the epilogue (norm scaling, compat
    weights, unary add, 2-label softmax as sigmoid). New Q rows are
    AllGathered (102 KB) for the next iteration.
"""

import sys
import os

if "/opt/trn_rl_repo" not in sys.path:
    sys.path.insert(0, "/opt/trn_rl_repo")

import numpy as np

import concourse.bass as bass
import concourse.tile as tile
from concourse import bacc, mybir
from concourse.bass_utils import run_bass_kernel_spmd

# ----- problem constants (hardcoded per the harness contract) -----
B, H, W = 2, 80, 80
N = H * W                 # 6400
P = 128                   # SBUF partitions
NT = N // P               # 50 j-tiles
N_CORES = 8
CHUNK = N // N_CORES      # 800 i's per core
NIC = (CHUNK + P - 1) // P  # 7 i-chunks (6x128 + 32)
I_CHUNKS = [(q * P, min(P, CHUNK - q * P)) for q in range(NIC)]

SXY_G, COMPAT_G = 3.0, 3.0
SXY_B, SRGB_B, COMPAT_B = 50.0, 5.0, 10.0
CLIP = 1e-5

F32 = mybir.dt.float32
# K storage / matmul operand dtype. fp16 keeps ~1e-3 overall accuracy
# (K in [0,1], messages accumulate in fp32 PSUM) at half the HBM traffic.
KDT = mybir.dt.float16
KDT_NP = np.float16

RG = [list(range(N_CORES))]

_PROGRAM_CACHE: dict = {}
LAST_RESULT = None  # BassKernelResults of the most recent run (for test.py)


def _build_program(iters: int):
    nc = bacc.Bacc(
        "TRN2", target_bir_lowering=False, debug=False, num_devices=N_CORES
    )

    # ---- I/O tensors ----
    def inp(name, shape):
        return nc.dram_tensor(name, list(shape), F32, kind="ExternalInput").ap()

    Lg = inp("Lg", (3, N))
    Rg = inp("Rg", (3, CHUNK))
    biasg = inp("biasg", (N,))
    Lb0 = inp("Lb0", (6, N))
    Rb0 = inp("Rb0", (6, CHUNK))
    biasb0 = inp("biasb0", (N,))
    Lb1 = inp("Lb1", (6, N))
    Rb1 = inp("Rb1", (6, CHUNK))
    biasb1 = inp("biasb1", (N,))
    negU = inp("negU", (CHUNK, 4))
    qinit = inp("qinit", (N, 4))
    ident = inp("ident", (P, P))
    qout = nc.dram_tensor("qout", [CHUNK, 2], F32, kind="ExternalOutput").ap()

    AF = mybir.ActivationFunctionType
    OP = mybir.AluOpType

    with tile.TileContext(nc) as tc:
        with (
            tc.tile_pool(name="const", bufs=1) as cpool,
            tc.tile_pool(name="dram", bufs=1, space="DRAM") as dpool,
        ):
            # ---------- persistent SBUF constants ----------
            Lg_sb = cpool.tile([3, N], F32)
            Lb0_sb = cpool.tile([6, N], F32)
            Lb1_sb = cpool.tile([6, N], F32)
            Rg_sb = cpool.tile([3, CHUNK], F32)
            Rb0_sb = cpool.tile([6, CHUNK], F32)
            Rb1_sb = cpool.tile([6, CHUNK], F32)
            biasg_sb = cpool.tile([P, NT], F32)
            biasb0_sb = cpool.tile([P, NT], F32)
            biasb1_sb = cpool.tile([P, NT], F32)
            negU_sb = cpool.tile([P, NIC, 4], F32)
            ident_sb = cpool.tile([P, P], F32)
            ones4 = cpool.tile([P, 4], F32)

            nc.sync.dma_start(Lg_sb[:], Lg)
            nc.sync.dma_start(Lb0_sb[:], Lb0)
            nc.sync.dma_start(Lb1_sb[:], Lb1)
            nc.sync.dma_start(Rg_sb[:], Rg)
            nc.sync.dma_start(Rb0_sb[:], Rb0)
            nc.sync.dma_start(Rb1_sb[:], Rb1)
            nc.sync.dma_start(
                biasg_sb[:], biasg.rearrange("(t p) -> p t", p=P)
            )
            nc.sync.dma_start(
                biasb0_sb[:], biasb0.rearrange("(t p) -> p t", p=P)
            )
            nc.sync.dma_start(
                biasb1_sb[:], biasb1.rearrange("(t p) -> p t", p=P)
            )
            nc.sync.dma_start(
                negU_sb[:, 0:6, :],
                negU[0 : 6 * P, :].rearrange("(q p) c -> p q c", p=P),
            )
            nc.sync.dma_start(negU_sb[0:32, 6, :], negU[6 * P : CHUNK, :])
            nc.sync.dma_start(ident_sb[:], ident)
            nc.vector.memset(ones4[:], 1.0)

            # row-sum accumulators (written column-by-column by build Exp)
            rowsum_g = cpool.tile([P, NT], F32)
            rowsum_b0 = cpool.tile([P, NT], F32)
            rowsum_b1 = cpool.tile([P, NT], F32)

            # K storage in HBM, tile-major [NT, P, CHUNK]
            KG = dpool.tile([NT, P, CHUNK], KDT, tag="KG")
            KB0 = dpool.tile([NT, P, CHUNK], KDT, tag="KB0")
            KB1 = dpool.tile([NT, P, CHUNK], KDT, tag="KB1")

            kernels = [
                (Lg_sb, Rg_sb, biasg_sb, KG, rowsum_g, "g"),
                (Lb0_sb, Rb0_sb, biasb0_sb, KB0, rowsum_b0, "b0"),
                (Lb1_sb, Rb1_sb, biasb1_sb, KB1, rowsum_b1, "b1"),
            ]

            # ---------- build phase ----------
            with (
                tc.tile_pool(name="bpsum", bufs=4, space="PSUM") as bpsum,
                tc.tile_pool(name="bk", bufs=4) as bkpool,
            ):
                for t in range(NT):
                    for L_sb, R_sb, bias_sb, KD, rsum, knm in kernels:
                        cdim = L_sb.shape[0]
                        ps = bpsum.tile([P, CHUNK], F32, tag="ps", name=f"ps{knm}{t}")
                        nc.tensor.matmul(
                            ps[:, 0:512],
                            lhsT=L_sb[:, bass.ts(t, P)],
                            rhs=R_sb[:, 0:512],
                            start=True,
                            stop=True,
                        )
                        nc.tensor.matmul(
                            ps[:, 512:CHUNK],
                            lhsT=L_sb[:, bass.ts(t, P)],
                            rhs=R_sb[:, 512:CHUNK],
                            start=True,
                            stop=True,
                        )
                        kt = bkpool.tile([P, CHUNK], KDT, tag="kt", name=f"kt{knm}{t}")
                        nc.scalar.activation(
                            kt[:],
                            ps[:],
                            AF.Exp,
                            bias=bias_sb[:, t : t + 1],
                            scale=1.0,
                            accum_out=rsum[:, t : t + 1],
                        )
                        nc.sync.dma_start(KD[t], kt[:])

            # ---------- normalization collectives ----------
            rs_in = dpool.tile([3, N], F32, tag="rs_in")
            rs_out = dpool.tile([3, N], F32, tag="rs_out")
            rs2_in = dpool.tile([N_CORES, 3, CHUNK], F32, tag="rs2_in")
            rs2_out = dpool.tile([3, CHUNK], F32, tag="rs2_out")

            for k, rsum in enumerate((rowsum_g, rowsum_b0, rowsum_b1)):
                nc.sync.dma_start(
                    rs_in[k].rearrange("(t p) -> p t", p=P), rsum[:]
                )
            # rank-major reshuffle of the partials for the ReduceScatter
            nc.sync.dma_start(
                rs2_in[:], rs_in.rearrange("k (r m) -> r k m", r=N_CORES)
            )
            nc.gpsimd.collective_compute(
                "AllReduce",
                OP.add,
                replica_groups=RG,
                ins=[rs_in.opt()],
                outs=[rs_out.opt()],
            )
            nc.gpsimd.collective_compute(
                "ReduceScatter",
                OP.add,
                replica_groups=RG,
                ins=[rs2_in.opt()],
                outs=[rs2_out.opt()],
            )

            # ---------- norms ----------
            # j-layout norms, replicated over the 4 Q columns for the
            # per-iteration rhs scaling:
            #   ngrep[:, t, c] = 1/sqrt(sum_g)   (c = 0..3)
            #   nbrep[:, t, 0:2] = 1/sqrt(sum_b0); [:, 2:4] = 1/sqrt(sum_b1)
            sums_j = cpool.tile([P, 3 * NT], F32)
            for k in range(3):
                nc.sync.dma_start(
                    sums_j[:, k * NT : (k + 1) * NT],
                    rs_out[k].rearrange("(t p) -> p t", p=P),
                )
            recip_j = cpool.tile([P, 3 * NT], F32)
            nc.vector.reciprocal(recip_j[:], sums_j[:])
            normj = cpool.tile([P, 3 * NT], F32)
            nc.scalar.activation(normj[:], recip_j[:], AF.Sqrt)

            ngrep = cpool.tile([P, NT, 4], F32)
            nbrep = cpool.tile([P, NT, 4], F32)
            for t in range(NT):
                nc.vector.tensor_scalar(
                    ngrep[:, t, :], ones4[:], normj[:, t : t + 1], None, OP.mult
                )
                nc.vector.tensor_scalar(
                    nbrep[:, t, 0:2],
                    ones4[:, 0:2],
                    normj[:, NT + t : NT + t + 1],
                    None,
                    OP.mult,
                )
                nc.vector.tensor_scalar(
                    nbrep[:, t, 2:4],
                    ones4[:, 0:2],
                    normj[:, 2 * NT + t : 2 * NT + t + 1],
                    None,
                    OP.mult,
                )

            # i-layout norms for this core's block: [P, 3*NIC]
            sums_i = cpool.tile([P, 3 * NIC], F32)
            nc.vector.memset(sums_i[:], 1.0)
            for k in range(3):
                nc.sync.dma_start(
                    sums_i[:, k * NIC : k * NIC + 6],
                    rs2_out[k, 0 : 6 * P].rearrange("(q p) -> p q", p=P),
                )
                nc.sync.dma_start(
                    sums_i[0:32, k * NIC + 6], rs2_out[k, 6 * P : CHUNK]
                )
            recip_i = cpool.tile([P, 3 * NIC], F32)
            nc.vector.reciprocal(recip_i[:], sums_i[:])
            normi = cpool.tile([P, 3 * NIC], F32)
            nc.scalar.activation(normi[:], recip_i[:], AF.Sqrt)

            # ---------- mean-field iterations ----------
            with (
                tc.tile_pool(name="kstream", bufs=3) as kpool,
                tc.tile_pool(name="acc", bufs=1, space="PSUM") as accpool,
                tc.tile_pool(name="tr", bufs=2, space="PSUM") as trpool,
                tc.tile_pool(name="ep", bufs=6) as eppool,
                tc.tile_pool(name="q", bufs=2) as qpool,
            ):
                qsrc = qinit
                for it in range(iters):
                    last = it == iters - 1

                    q_all = qpool.tile([P, NT, 4], F32, tag="q_all", name=f"q_all{it}")
                    nc.sync.dma_start(
                        q_all[:], qsrc.rearrange("(t p) c -> p t c", p=P)
                    )
                    rhs_g = qpool.tile([P, NT, 4], KDT, tag="rhs_g", name=f"rhs_g{it}")
                    rhs_b = qpool.tile([P, NT, 4], KDT, tag="rhs_b", name=f"rhs_b{it}")
                    nc.vector.tensor_tensor(rhs_g[:], q_all[:], ngrep[:], OP.mult)
                    nc.vector.tensor_tensor(rhs_b[:], q_all[:], nbrep[:], OP.mult)

                    pg = accpool.tile([4, CHUNK], F32, tag="pg", name=f"pg{it}")
                    pb0 = accpool.tile([2, CHUNK], F32, tag="pb0", name=f"pb0{it}")
                    pb1 = accpool.tile([2, CHUNK], F32, tag="pb1", name=f"pb1{it}")

                    for t in range(NT):
                        kg = kpool.tile([P, CHUNK], KDT, tag="kg", name=f"kg{it}_{t}")
                        kb0 = kpool.tile([P, CHUNK], KDT, tag="kb0", name=f"kb0{it}_{t}")
                        kb1 = kpool.tile([P, CHUNK], KDT, tag="kb1", name=f"kb1{it}_{t}")
                        nc.sync.dma_start(kg[:], KG[t])
                        nc.sync.dma_start(kb0[:], KB0[t])
                        nc.sync.dma_start(kb1[:], KB1[t])
                        st = dict(start=(t == 0), stop=(t == NT - 1))
                        for c0, cn in ((0, 512), (512, CHUNK - 512)):
                            nc.tensor.matmul(
                                pg[:, c0 : c0 + cn],
                                lhsT=rhs_g[:, t, :],
                                rhs=kg[:, c0 : c0 + cn],
                                **st,
                            )
                            nc.tensor.matmul(
                                pb0[:, c0 : c0 + cn],
                                lhsT=rhs_b[:, t, 0:2],
                                rhs=kb0[:, c0 : c0 + cn],
                                **st,
                            )
                            nc.tensor.matmul(
                                pb1[:, c0 : c0 + cn],
                                lhsT=rhs_b[:, t, 2:4],
                                rhs=kb1[:, c0 : c0 + cn],
                                **st,
                            )

                    # epilogue: PSUM -> SBUF, transpose to [i, l], softmax
                    sg = eppool.tile([4, CHUNK], F32, tag="sg", name=f"sg{it}")
                    sb0 = eppool.tile([2, CHUNK], F32, tag="sb0", name=f"sb0{it}")
                    sb1 = eppool.tile([2, CHUNK], F32, tag="sb1", name=f"sb1{it}")
                    nc.vector.tensor_copy(sg[:], pg[:])
                    nc.vector.tensor_copy(sb0[:], pb0[:])
                    nc.vector.tensor_copy(sb1[:], pb1[:])

                    qstage = qpool.tile(
                        [P, NIC, 4], F32, tag="qstage", name=f"qstage{it}"
                    )
                    for q, (i0, iw) in enumerate(I_CHUNKS):
                        tr = trpool.tile([P, 8], F32, tag="tr", name=f"tr{it}_{q}")
                        nc.tensor.transpose(
                            tr[0:iw, 0:4], sg[:, i0 : i0 + iw], ident_sb[0:4, 0:4]
                        )
                        nc.tensor.transpose(
                            tr[0:iw, 4:6], sb0[:, i0 : i0 + iw], ident_sb[0:2, 0:2]
                        )
                        nc.tensor.transpose(
                            tr[0:iw, 6:8], sb1[:, i0 : i0 + iw], ident_sb[0:2, 0:2]
                        )
                        ag = eppool.tile([P, 4], F32, tag="ag", name=f"ag{it}_{q}")
                        ab = eppool.tile([P, 4], F32, tag="ab", name=f"ab{it}_{q}")
                        # compat * norm_i * message
                        nc.vector.tensor_scalar(
                            ag[0:iw, :],
                            tr[0:iw, 0:4],
                            normi[0:iw, q : q + 1],
                            COMPAT_G,
                            OP.mult,
                            OP.mult,
                        )
                        nc.vector.tensor_scalar(
                            ab[0:iw, 0:2],
                            tr[0:iw, 4:6],
                            normi[0:iw, NIC + q : NIC + q + 1],
                            COMPAT_B,
                            OP.mult,
                            OP.mult,
                        )
                        nc.vector.tensor_scalar(
                            ab[0:iw, 2:4],
                            tr[0:iw, 6:8],
                            normi[0:iw, 2 * NIC + q : 2 * NIC + q + 1],
                            COMPAT_B,
                            OP.mult,
                            OP.mult,
                        )
                        s = eppool.tile([P, 4], F32, tag="s", name=f"s{it}_{q}")
                        nc.vector.tensor_tensor(
                            s[0:iw, :], ag[0:iw, :], ab[0:iw, :], OP.add
                        )
                        nc.vector.tensor_tensor(
                            s[0:iw, :], s[0:iw, :], negU_sb[0:iw, q, :], OP.add
                        )
                        d = eppool.tile([P, 2], F32, tag="d", name=f"d{it}_{q}")
                        nc.vector.tensor_tensor(
                            d[0:iw, 0:1], s[0:iw, 0:1], s[0:iw, 1:2], OP.subtract
                        )
                        nc.vector.tensor_tensor(
                            d[0:iw, 1:2], s[0:iw, 2:3], s[0:iw, 3:4], OP.subtract
                        )
                        if last:
                            nc.scalar.activation(
                                qstage[0:iw, q, 0:1], d[0:iw, 0:1], AF.Sigmoid
                            )
                            nc.scalar.activation(
                                qstage[0:iw, q, 1:2], d[0:iw, 1:2], AF.Sigmoid
                            )
                        else:
                            nc.scalar.activation(
                                qstage[0:iw, q, 0:1], d[0:iw, 0:1], AF.Sigmoid
                            )
                            nc.scalar.activation(
                                qstage[0:iw, q, 1:2], d[0:iw, 0:1], AF.Sigmoid,
                                scale=-1.0,
                            )
                            nc.scalar.activation(
                                qstage[0:iw, q, 2:3], d[0:iw, 1:2], AF.Sigmoid
                            )
                            nc.scalar.activation(
                                qstage[0:iw, q, 3:4], d[0:iw, 1:2], AF.Sigmoid,
                                scale=-1.0,
                            )

                    if last:
                        nc.sync.dma_start(
                            qout[0 : 6 * P, :].rearrange("(q p) c -> p q c", p=P),
                            qstage[:, 0:6, 0:2],
                        )
                        nc.sync.dma_start(
                            qout[6 * P : CHUNK, :], qstage[0:32, 6, 0:2]
                        )
                    else:
                        qag_in = dpool.tile(
                            [CHUNK, 4], F32, tag=f"qag_in{it}", name=f"qag_in{it}"
                        )
                        qag_out = dpool.tile(
                            [N, 4], F32, tag=f"qag_out{it}", name=f"qag_out{it}"
                        )
                        nc.sync.dma_start(
                            qag_in[0 : 6 * P, :].rearrange("(q p) c -> p q c", p=P),
                            qstage[:, 0:6, :],
                        )
                        nc.sync.dma_start(
                            qag_in[6 * P : CHUNK, :], qstage[0:32, 6, :]
                        )
                        nc.gpsimd.collective_compute(
                            "AllGather",
                            OP.bypass,
                            replica_groups=RG,
                            ins=[qag_in.opt()],
                            outs=[qag_out.opt()],
                        )
                        qsrc = qag_out

    nc.compile()
    return nc


def _host_inputs(img: np.ndarray, pred: np.ndarray):
    """Per-core input dicts (float32), computed in float64 on host."""
    yy, xx = np.mgrid[0:H, 0:W]
    pos = np.stack([xx.ravel(), yy.ravel()], 1).astype(np.float64)  # [N,2] (x,y)
    colors = img.reshape(B, 3, N).transpose(0, 2, 1).astype(np.float64) * 255.0

    fg = pos / SXY_G                       # [N,2]
    q2g = 0.5 * (fg * fg).sum(1)           # [N]
    Lg = np.concatenate([fg.T, np.ones((1, N))], 0)

    fb = [
        np.concatenate([pos / SXY_B, colors[b] / SRGB_B], 1) for b in range(B)
    ]                                      # [N,5] each
    q2b = [0.5 * (f * f).sum(1) for f in fb]
    Lb = [np.concatenate([fb[b].T, np.ones((1, N))], 0) for b in range(B)]

    p = pred.reshape(B, N).astype(np.float64)
    probs = np.clip(np.stack([p, 1.0 - p], -1), CLIP, 1.0)  # [B,N,2]
    negU = np.log(probs)
    Q0 = probs / probs.sum(-1, keepdims=True)
    qinit = np.concatenate([Q0[0], Q0[1]], 1)  # [N,4], c = 2*img + l

    ident = np.eye(P, dtype=np.float32)

    f32 = lambda a: np.ascontiguousarray(a, dtype=np.float32)
    shared = {
        "Lg": f32(Lg),
        "biasg": f32(-q2g),
        "Lb0": f32(Lb[0]),
        "biasb0": f32(-q2b[0]),
        "Lb1": f32(Lb[1]),
        "biasb1": f32(-q2b[1]),
        "qinit": f32(qinit),
        "ident": ident,
    }
    in_maps = []
    for c in range(N_CORES):
        sl = slice(c * CHUNK, (c + 1) * CHUNK)
        m = dict(shared)
        m["Rg"] = f32(np.concatenate([fg[sl].T, -q2g[None, sl]], 0))
        m["Rb0"] = f32(
            np.concatenate([fb[0][sl].T, -q2b[0][None, sl]], 0)
        )
        m["Rb1"] = f32(
            np.concatenate([fb[1][sl].T, -q2b[1][None, sl]], 0)
        )
        m["negU"] = f32(
            np.concatenate([negU[0, sl], negU[1, sl]], 1)
        )
        in_maps.append(m)
    return in_maps, Q0


def kernel(img, pred, iters):
    global LAST_RESULT
    img = np.asarray(img, dtype=np.float32)
    pred = np.asarray(pred, dtype=np.float32)
    iters = int(np.asarray(iters))

    in_maps, Q0 = _host_inputs(img, pred)

    if iters <= 0:
        out = Q0[..., 0].astype(np.float32).reshape(B, 1, H, W)
        return out

    if iters not in _PROGRAM_CACHE:
        _PROGRAM_CACHE[iters] = _build_program(iters)
    nc = _PROGRAM_CACHE[iters]

    trace = bool(int(os.environ.get("BASS_CRF_TRACE", "0")))
    res = run_bass_kernel_spmd(
        nc, in_maps, core_ids=list(range(N_CORES)), trace=trace
    )
    LAST_RESULT = res

    prob0 = np.concatenate(
        [res.results[c]["qout"] for c in range(N_CORES)], axis=0
    )  # [N, 2], columns = image index
    out = np.stack(
        [prob0[:, 0].reshape(1, H, W), prob0[:, 1].reshape(1, H, W)], axis=0
    ).astype(np.float32)
    return out
